# revision 46
# baseline (speedup 1.0000x reference)
"""Contextual loss kernel for Trainium2 (Bass/Tile), 8 NeuronCores.

Reference computation (per batch b, B=4, C=128, N=64*64=4096):
  mean_y[c] = spatial mean of feature_y
  fx,fy centered by mean_y; columns L2-normalized over channels
  S[n,m]    = <fxn[:,n], fyn[:,m]>           (cosine similarity)
  d = 1-S;  d_norm = d / (min_m d + 1e-3);  w = exp((1-d_norm)/h);  A = w/sum_m w
  CX[b] = mean_n max_m A;  loss = -log(CX)

Per-row identity used on device (with Smax = max_m S, c = 1/(h*(1-Smax+eps))):
  max_m A = 1 / sum_m exp(c*(S[m]-Smax))

x-normalization is folded into the activation scale: G = (x-mu).y_hat,
S = G/nx with nx = ||x-mu||+1e-10.  The exp pass uses
scale = 1/(H*((1+eps)*nx - Gmax)) and NO bias (args span ~[-7,7], safe in
f32); maxA = exp(scale*Gmax)/sum_m exp(scale*G_m) applies the numerator
once in the tail (one batched [P,16] exp).

Sharding: 8 cores = 4 batches x 2 row-halves. Each core gets its half of
feature_x's rows ([2048,128]) plus the full feature_y ([4096,128]) of its
batch, computes sum_rows 1/r locally; host combines and takes -log.

Main loop: two interleaved matmul passes with recompute (pass1 row-max
on PE+DVE, pass2 exp on PE+ACT, disjoint 4-bank PSUM pools), staggered
TWO blocks apart: pass2(rb-2) is emitted after pass1(rb), so the
per-block scale chain (gpsimd mul/add + DVE reciprocal) is fully hidden
and ACT runs the 64 [128,1024] exp+accum pairs back-to-back (measured
99.7% ACT busy over the ~82us loop).  Block 0's first two pass2
quarter-matmuls are hoisted into the pipeline fill so the first exp
waits only on scl(0).  (A single-pass variant using
TENSOR_TENSOR_REDUCE for fused copy+max faults at runtime on this
firmware — the opcode compiles but the engine dies — so the recompute
scheme stands; PE has the headroom.)

Prologue: y is DMA'd in interleaved 4-tile subchunks on the two HWDGE
queues so each 8-tile cast group completes early; the channel mean is
PE-accumulated from the bf16 casts (1-pass matmuls, vs 2-pass f32);
center/norm runs per 8-tile group (DVE 2x on bf16), the normalize
multiply runs on gpsimd (otherwise idle; frees the DVE-serial chain),
and per-group transposes alternate HWDGE queues.  Dep-free dummy
matmuls keep the PE p-state up through the DMA/mean window (a cold PE
runs 512-wide matmuls ~2x slow).  The Exp activation table preload is
ordered after every Sqrt via an explicit dependency tile, so no 1.3us
table reload lands on the loop's critical path.  The output DMA rides
the gpsimd SWDGE queue (its ~7us completion latency is the tail floor
either way).
"""

import numpy as np

import concourse.bacc as bacc
import concourse.bass as bass
import concourse.tile as tile
from concourse import masks, mybir
from concourse.bass_utils import run_bass_kernel_spmd

F32 = mybir.dt.float32
BF16 = mybir.dt.bfloat16
AF = mybir.ActivationFunctionType
ALU = mybir.AluOpType
AX = mybir.AxisListType

B = 4
C = 128
N = 4096          # spatial positions per batch
ROWS = N // 2     # rows of S per core (x-half)
P = 128           # partitions
NYT = N // P      # 32 y tiles
NXT = ROWS // P   # 16 x tiles
CHUNK = 512       # matmul free dim (one PSUM bank)
NRB = ROWS // P   # 16 row blocks per core
NG = 4            # y pipeline groups (8 tiles each)
GT = NYT // NG    # tiles per group

H_PARAM = 0.1
EPS_MIN = 0.001
EPS_NORM = 1e-10


def build_nc():
    nc = bacc.Bacc(None)
    fx = nc.declare_dram_parameter("fx", [ROWS, C], F32, isOutput=False)
    fy = nc.declare_dram_parameter("fy", [N, C], F32, isOutput=False)
    # raw per-block outputs (quarter exp-sums, row maxes, scales); the
    # final maxA/reduction math runs on the host in f64.  Shipping these
    # immediately after the last accumulator read (instead of running a
    # ~1.6us on-device tail chain first) starts the output DMA's fixed
    # ~7us completion wait sooner.
    part = nc.declare_dram_parameter("part", [P, 6 * NRB], F32, isOutput=True)

    # contiguous per-partition DMA mapping: partition p takes rows
    # [p*T, (p+1)*T).  This permutes S rows/columns vs the reference,
    # which is harmless: every reduction (max/sum) is along full rows.
    fy_t = fy.rearrange("(p i) c -> p i c", p=P)   # [128, 32, 128]
    fx_t = fx.rearrange("(p i) c -> p i c", p=P)   # [128, 16, 128]

    with tile.TileContext(nc) as tc:
        with (
            tc.tile_pool(name="singles", bufs=1) as singles,
            tc.tile_pool(name="raw", bufs=1) as raw,
            tc.tile_pool(name="tmats", bufs=1) as tmats,
            tc.tile_pool(name="stat", bufs=6) as stat,
        ):
            # ---- constants ----
            ones_bf = singles.tile([P, P], BF16)
            nc.vector.memset(ones_bf[:], 1.0)
            negh_col = singles.tile([P, 1], F32)    # -H (gpsimd chain const)
            nc.vector.memset(negh_col[:], -H_PARAM)
            dummy = singles.tile([P, 1], F32)
            nc.vector.memset(dummy[:], 0.0)
            mean_bf = singles.tile([P, C], BF16)
            # preload the Sqrt table set during the DMA phase (dep-free;
            # Square also lives in that set, so no reload before sq)
            nc.scalar.activation(out=dummy[:], in_=dummy[:], func=AF.Sqrt)

            nsy = singles.tile([P, NYT], F32)
            nsx = singles.tile([P, NXT], F32)
            sdy = singles.tile([P, NYT], F32)
            sdx = singles.tile([P, NXT], F32)
            invy = singles.tile([P, NYT], F32)
            nxh = singles.tile([P, NXT], F32)       # H*(1+eps)*nx
            scl_all = singles.tile([P, NRB], F32)   # c/nx per block
            gmax_all = singles.tile([P, NRB], F32)  # row max of G per block
            rq_all = singles.tile([P, NRB, 4], F32)  # quarter sums

            # ---- load inputs (y first: the mean gates everything).
            # 8-tile chunks = 4KB-contiguous per-partition descriptors (2KB
            # descriptors measurably cut DMA rate); y takes both HWDGE
            # queues first, x strictly after.  Chunk k on each queue covers
            # groups so the mean (which needs ALL of y) completes earliest.
            ysp = raw.tile([P, NYT, C], F32)
            xsp = raw.tile([P, NXT, C], F32)
            nc.sync.dma_start(out=ysp[:, 0:8, :], in_=fy_t[:, 0:8, :])
            nc.scalar.dma_start(out=ysp[:, 8:16, :], in_=fy_t[:, 8:16, :])
            nc.sync.dma_start(out=ysp[:, 16:24, :], in_=fy_t[:, 16:24, :])
            nc.scalar.dma_start(out=ysp[:, 24:32, :], in_=fy_t[:, 24:32, :])
            nc.sync.dma_start(out=xsp[:, 0:8, :], in_=fx_t[:, 0:8, :])
            nc.scalar.dma_start(out=xsp[:, 8:16, :], in_=fx_t[:, 8:16, :])

            # PE warmup: dep-free dummy matmuls while the DMA runs, so the
            # PE reaches full p-state before the first real block matmuls
            # (cold PE runs 512-wide matmuls ~2x slow).
            with tc.tile_pool(name="ps_warm", bufs=1,
                              space=bass.MemorySpace.PSUM) as warm_pool:
                warm = warm_pool.tile([P, CHUNK], F32)
                wsrc = singles.tile([P, CHUNK], BF16)
                nc.gpsimd.memset(wsrc[:], 0.0)
                for _ in range(24):
                    nc.tensor.matmul(warm[:], ones_bf[:], wsrc[:],
                                     start=True, stop=True)

            # bf16 casts per 8-tile DMA chunk (feed both the mean matmuls
            # and the centering chain)
            ybf_raw = raw.tile([P, NYT, C], BF16)
            xbf_raw = raw.tile([P, NXT, C], BF16)
            for h in range(NG):
                a = h * GT
                nc.vector.tensor_copy(ybf_raw[:, a:a + GT, :],
                                      ysp[:, a:a + GT, :])
            for j in range(2):
                nc.vector.tensor_copy(xbf_raw[:, j * 8:(j + 1) * 8, :],
                                      xsp[:, j * 8:(j + 1) * 8, :])

            # ---- mean over y's spatial axis: PE-accumulated from the bf16
            # casts as they land (1 cyc/row vs 2 for f32).  ones[P,P] @ tile
            # sums partitions; accumulating all 32 tiles sums tiles too.
            with tc.tile_pool(name="ps_bc", bufs=1,
                              space=bass.MemorySpace.PSUM) as ps_bc_pool:
                ps_bc = ps_bc_pool.tile([P, C], F32)
                for t in range(NYT):
                    nc.tensor.matmul(ps_bc[:], ones_bf[:], ybf_raw[:, t, :],
                                     start=(t == 0), stop=(t == NYT - 1))
                nc.scalar.mul(mean_bf[:], ps_bc[:], 1.0 / N)

            # second PE warmup batch: keeps the PE clocked up between the
            # mean matmuls and the first block matmuls (~20-31us window)
            with tc.tile_pool(name="ps_warm2", bufs=1,
                              space=bass.MemorySpace.PSUM) as warm_pool2:
                warm2 = warm_pool2.tile([P, CHUNK], F32)
                for _ in range(30):
                    nc.tensor.matmul(warm2[:], ones_bf[:], wsrc[:],
                                     start=True, stop=True)

            mean_g = mean_bf[:].rearrange("p (u c) -> p u c", u=1)

            def mbc(k):
                return mean_g.broadcast_to([P, k, C])

            # ---- y chain per 8-tile group: center, norm, normalize,
            # transpose.  All DVE ops are bf16 (2x mode).  The x subtract +
            # transpose slot in after group 0 (xt is needed by the first
            # block matmuls, but must not delay y group 0).
            xbf = raw.tile([P, NXT, C], BF16)
            sqx = raw.tile([P, NXT, C], F32)
            xt = tmats.tile([P, NXT, P], BF16)
            ybf = raw.tile([P, NYT, C], BF16)
            sq = raw.tile([P, NYT, C], F32)
            yt = tmats.tile([P, N], BF16)        # [c, (t q)] channel-major
            ytv = yt[:].rearrange("p (t q) -> p t q", q=P)
            ivg = invy[:].rearrange("p (t u) -> p t u", u=1)
            for g in range(NG):
                a = g * GT
                nc.vector.tensor_sub(ybf[:, a:a + GT, :],
                                     ybf_raw[:, a:a + GT, :], mbc(GT))
                nc.scalar.activation(out=sq[:, a:a + GT, :],
                                     in_=ybf[:, a:a + GT, :], func=AF.Square)
                nc.vector.reduce_sum(nsy[:, a:a + GT], sq[:, a:a + GT, :],
                                     axis=AX.X)
                nc.scalar.activation(sdy[:, a:a + GT], nsy[:, a:a + GT],
                                     AF.Sqrt)
                nc.vector.reciprocal(invy[:, a:a + GT], sdy[:, a:a + GT])
                # normalize on gpsimd: takes the biggest per-group op off
                # the DVE-serial critical chain (Pool is otherwise idle).
                # NOTE: keep ALL groups on one engine — splitting ybf's
                # writers across DVE and Pool produced an intermittent nan
                # on HW (and no speedup); mixed-engine writers to one tile
                # are not worth the risk.
                nc.gpsimd.tensor_mul(
                    ybf[:, a:a + GT, :], ybf[:, a:a + GT, :],
                    ivg[:, a:a + GT].broadcast_to([P, GT, C]))
                (nc.sync if g % 2 == 0 else nc.scalar).dma_start_transpose(
                    out=ytv[:, a:a + GT, :],
                    in_=ybf[:, a:a + GT, :].rearrange("p t c -> p (t c)"))
                if g == 0:
                    nc.vector.tensor_sub(xbf[:], xbf_raw[:], mbc(NXT))
                    nc.sync.dma_start_transpose(
                        out=xt[:],
                        in_=xbf[:].rearrange("p t c -> p (t c)"))

            # x norms after the y chain: they only feed the scale chain
            # (needed ~when block 0's maxes finish), and emitting them last
            # keeps the y-critical DVE stream unclogged.  The reduce is
            # split in 4 so no single 2.2us DVE op can wedge into the
            # y-critical window.
            nc.scalar.activation(out=sqx[:], in_=xbf[:], func=AF.Square)
            for j in range(4):
                nc.vector.reduce_sum(nsx[:, 4 * j:4 * (j + 1)],
                                     sqx[:, 4 * j:4 * (j + 1), :], axis=AX.X)
            nc.scalar.activation(sdx[:], nsx[:], AF.Sqrt)
            nc.vector.tensor_scalar_mul(nxh[:], sdx[:],
                                        H_PARAM * (1.0 + EPS_MIN))

            # preload the Exp activation table strictly after every Sqrt:
            # the dep tile mixes all four y-group sqrts and the x sqrt so
            # the scheduler cannot reorder any sqrt past the preload (a
            # reload on the loop's critical path costs 1.3us).
            sqdep = stat.tile([P, 1], F32, tag="sqdep", name="sqdep")
            nc.vector.tensor_add(sqdep[:], sdy[:, GT - 1:GT],
                                 sdy[:, 2 * GT - 1:2 * GT])
            for g in range(2, NG):
                nc.vector.tensor_add(sqdep[:], sqdep[:],
                                     sdy[:, (g + 1) * GT - 1:(g + 1) * GT])
            nc.vector.tensor_add(sqdep[:], sqdep[:], sdx[:, NXT - 1:NXT])
            # scale=0: exp(0)=1 regardless of the dep-sum's magnitude (the
            # sum of sqrts can overflow exp for finer group counts)
            nc.scalar.activation(out=dummy[:], in_=sqdep[:], func=AF.Exp,
                                 scale=0.0)

            def yrhs(j):       # 512-wide chunk j of the y matrix [C, N]
                return yt[:, CHUNK * j:CHUNK * (j + 1)]

            QUART = 1024
            NQ = N // QUART

            # ---- main loop: two interleaved passes with recompute
            # (pass1 max on PE+DVE, pass2 exp on PE+ACT, disjoint PSUM
            # halves, two blocks apart so the per-block scale chain
            # (gpsimd mul/add + DVE reciprocal) never stalls ACT).
            def pass1(rb, pool):
                lhs = xt[:, rb, :]
                mxq = stat.tile([P, NQ], F32, tag="mxq", name="mxq")
                for q in range(NQ):
                    ps = pool.tile([P, QUART], F32, tag="p1", name="ps1")
                    for j in range(2):
                        nc.tensor.matmul(
                            ps[:, j * CHUNK:(j + 1) * CHUNK],
                            lhs, yrhs(2 * q + j), start=True, stop=True)
                    nc.vector.reduce_max(mxq[:, q:q + 1], ps[:],
                                         axis=AX.X)
                gmax = gmax_all[:, rb:rb + 1]
                nc.vector.reduce_max(gmax, mxq[:], axis=AX.X)
                # tden = nxh - H*Gmax ; scl = 1/tden.  No bias: exp args
                # span only ~[-7, 7], so the max-shift is unnecessary;
                # the numerator exp(c*Smax) is applied once in the tail.
                tden = stat.tile([P, 1], F32, tag="tden", name="tden")
                if rb == 0:
                    nc.vector.tensor_scalar(
                        out=tden[:], in0=gmax, scalar1=-H_PARAM,
                        scalar2=nxh[:, rb:rb + 1],
                        op0=ALU.mult, op1=ALU.add)
                else:
                    hg = stat.tile([P, 1], F32, tag="hg", name="hg")
                    nc.gpsimd.tensor_mul(hg[:], gmax, negh_col[:])
                    nc.gpsimd.tensor_add(tden[:], hg[:], nxh[:, rb:rb + 1])
                nc.vector.reciprocal(scl_all[:, rb:rb + 1], tden[:])

            def p2mm(rb, pool, q):
                lhs = xt[:, rb, :]
                ps = pool.tile([P, QUART], F32, tag="p2", name="ps2")
                for j in range(2):
                    nc.tensor.matmul(
                        ps[:, j * CHUNK:(j + 1) * CHUNK],
                        lhs, yrhs(2 * q + j), start=True, stop=True)
                return ps

            def p2exp(rb, ps, q):
                nc.scalar.activation(
                    out=ps[:], in_=ps[:], func=AF.Exp,
                    scale=scl_all[:, rb:rb + 1],
                    accum_out=rq_all[:, rb, q:q + 1])

            def pass2(rb, pool, pre=()):
                for q, ps in pre:
                    p2exp(rb, ps, q)
                for q in range(len(pre), NQ):
                    ps = p2mm(rb, pool, q)
                    p2exp(rb, ps, q)

            with (
                tc.tile_pool(name="ps_p1", bufs=2,
                             space=bass.MemorySpace.PSUM) as pool1,
                tc.tile_pool(name="ps_p2", bufs=2,
                             space=bass.MemorySpace.PSUM) as pool2,
            ):
                # fill: hoist block 0's first two quarter-matmuls so the
                # first exp waits only on scl(0), not a third PE pass
                pass1(0, pool1)
                pre0 = [(q, p2mm(0, pool2, q)) for q in range(2)]
                pass1(1, pool1)
                for rb in range(2, NRB + 2):
                    pass2(rb - 2, pool2, pre=pre0 if rb == 2 else ())
                    if rb < NRB:
                        pass1(rb, pool1)

            # ---- tail: ship raw ingredients on three parallel queues ----
            nc.gpsimd.dma_start(
                out=part[:, 0:4 * NRB],
                in_=rq_all[:].rearrange("p b q -> p (b q)"))
            nc.sync.dma_start(out=part[:, 4 * NRB:5 * NRB], in_=gmax_all[:])
            nc.scalar.dma_start(out=part[:, 5 * NRB:6 * NRB], in_=scl_all[:])

    nc.compile()
    return nc


_NC_CACHE = None


def _get_nc():
    global _NC_CACHE
    if _NC_CACHE is None:
        _NC_CACHE = build_nc()
    return _NC_CACHE


def _in_maps(feature_x, feature_y):
    fx = np.ascontiguousarray(
        np.asarray(feature_x, dtype=np.float32).reshape(B, N, C))
    fy = np.ascontiguousarray(
        np.asarray(feature_y, dtype=np.float32).reshape(B, N, C))
    maps = []
    for core in range(8):
        b, h = divmod(core, 2)
        maps.append({
            "fx": np.ascontiguousarray(fx[b, h * ROWS:(h + 1) * ROWS, :]),
            "fy": fy[b],
        })
    return maps


def _combine(results):
    # part[:, 0:64]=rq (per-block quarter exp-sums), [64:80]=Gmax,
    # [80:96]=scl.  maxA per block-row = exp(scl*Gmax) / sum_q rq.
    sums = []
    for r in results:
        part = np.asarray(r["part"], dtype=np.float64)
        rq = part[:, :4 * NRB].reshape(P, NRB, 4)
        gmax = part[:, 4 * NRB:5 * NRB]
        scl = part[:, 5 * NRB:6 * NRB]
        maxa = np.exp(scl * gmax) / rq.sum(axis=-1)
        sums.append(float(maxa.sum()))
    loss = np.empty(B, dtype=np.float64)
    for b in range(B):
        cx = (sums[2 * b] + sums[2 * b + 1]) / N
        loss[b] = -np.log(cx)
    return loss.astype(np.float32)


def kernel(feature_x, feature_y):
    nc = _get_nc()
    res = run_bass_kernel_spmd(nc, _in_maps(feature_x, feature_y),
                               core_ids=list(range(8)))
    return _combine(res.results)


def kernel_traced(feature_x, feature_y, **kwargs):
    """Like kernel() but with tracing; returns (loss, BassKernelResults)."""
    nc = _get_nc()
    res = run_bass_kernel_spmd(nc, _in_maps(feature_x, feature_y),
                               core_ids=list(range(8)), trace=True, **kwargs)
    return _combine(res.results), res


# revision 47
# speedup vs baseline: 1.0106x; 1.0106x over previous
"""Contextual loss kernel for Trainium2 (Bass/Tile), 8 NeuronCores.

Reference computation (per batch b, B=4, C=128, N=64*64=4096):
  mean_y[c] = spatial mean of feature_y
  fx,fy centered by mean_y; columns L2-normalized over channels
  S[n,m]    = <fxn[:,n], fyn[:,m]>           (cosine similarity)
  d = 1-S;  d_norm = d / (min_m d + 1e-3);  w = exp((1-d_norm)/h);  A = w/sum_m w
  CX[b] = mean_n max_m A;  loss = -log(CX)

Per-row identity used on device (with Smax = max_m S, c = 1/(h*(1-Smax+eps))):
  max_m A = 1 / sum_m exp(c*(S[m]-Smax))

x-normalization is folded into the activation scale: G = (x-mu).y_hat,
S = G/nx with nx = ||x-mu||+1e-10.  The exp pass uses
scale = 1/(H*((1+eps)*nx - Gmax)) and NO bias (args span ~[-7,7], safe in
f32); maxA = exp(scale*Gmax)/sum_m exp(scale*G_m) applies the numerator
once in the tail (one batched [P,16] exp).

Sharding: 8 cores = 4 batches x 2 row-halves. Each core gets its half of
feature_x's rows ([2048,128]) plus the full feature_y ([4096,128]) of its
batch, computes sum_rows 1/r locally; host combines and takes -log.

Main loop: two interleaved matmul passes with recompute (pass1 row-max
on PE+DVE, pass2 exp on PE+ACT, disjoint 4-bank PSUM pools), staggered
TWO blocks apart: pass2(rb-2) is emitted after pass1(rb), so the
per-block scale chain (gpsimd mul/add + DVE reciprocal) is fully hidden
and ACT runs the 64 [128,1024] exp+accum pairs back-to-back (measured
99.7% ACT busy over the ~82us loop).  Block 0's first two pass2
quarter-matmuls are hoisted into the pipeline fill so the first exp
waits only on scl(0).  (A single-pass variant using
TENSOR_TENSOR_REDUCE for fused copy+max faults at runtime on this
firmware — the opcode compiles but the engine dies — so the recompute
scheme stands; PE has the headroom.)

Prologue: y is DMA'd in interleaved 4-tile subchunks on the two HWDGE
queues so each 8-tile cast group completes early; the channel mean is
PE-accumulated from the bf16 casts (1-pass matmuls, vs 2-pass f32);
center/norm runs per 8-tile group (DVE 2x on bf16), the normalize
multiply runs on gpsimd (otherwise idle; frees the DVE-serial chain),
and per-group transposes alternate HWDGE queues.  Dep-free dummy
matmuls keep the PE p-state up through the DMA/mean window (a cold PE
runs 512-wide matmuls ~2x slow).  The Exp activation table preload is
ordered after every Sqrt via an explicit dependency tile, so no 1.3us
table reload lands on the loop's critical path.  The output DMA rides
the gpsimd SWDGE queue (its ~7us completion latency is the tail floor
either way).
"""

import numpy as np

import concourse.bacc as bacc
import concourse.bass as bass
import concourse.tile as tile
from concourse import masks, mybir
from concourse.bass_utils import run_bass_kernel_spmd

F32 = mybir.dt.float32
BF16 = mybir.dt.bfloat16
AF = mybir.ActivationFunctionType
ALU = mybir.AluOpType
AX = mybir.AxisListType

B = 4
C = 128
N = 4096          # spatial positions per batch
ROWS = N // 2     # rows of S per core (x-half)
P = 128           # partitions
NYT = N // P      # 32 y tiles
NXT = ROWS // P   # 16 x tiles
CHUNK = 512       # matmul free dim (one PSUM bank)
NRB = ROWS // P   # 16 row blocks per core
NG = 4            # y pipeline groups (8 tiles each)
GT = NYT // NG    # tiles per group

H_PARAM = 0.1
EPS_MIN = 0.001
EPS_NORM = 1e-10


def build_nc():
    nc = bacc.Bacc(None)
    fx = nc.declare_dram_parameter("fx", [ROWS, C], F32, isOutput=False)
    fy = nc.declare_dram_parameter("fy", [N, C], F32, isOutput=False)
    # raw per-block outputs (quarter exp-sums, row maxes, scales); the
    # final maxA/reduction math runs on the host in f64.  Shipping these
    # immediately after the last accumulator read (instead of running a
    # ~1.6us on-device tail chain first) starts the output DMA's fixed
    # ~7us completion wait sooner.
    part = nc.declare_dram_parameter("part", [P, 6 * NRB], F32, isOutput=True)

    # contiguous per-partition DMA mapping: partition p takes rows
    # [p*T, (p+1)*T).  This permutes S rows/columns vs the reference,
    # which is harmless: every reduction (max/sum) is along full rows.
    fy_t = fy.rearrange("(p i) c -> p i c", p=P)   # [128, 32, 128]
    fx_t = fx.rearrange("(p i) c -> p i c", p=P)   # [128, 16, 128]

    with tile.TileContext(nc) as tc:
        with (
            tc.tile_pool(name="singles", bufs=1) as singles,
            tc.tile_pool(name="raw", bufs=1) as raw,
            tc.tile_pool(name="tmats", bufs=1) as tmats,
            tc.tile_pool(name="stat", bufs=6) as stat,
        ):
            # ---- constants ----
            ones_bf = singles.tile([P, P], BF16)
            nc.vector.memset(ones_bf[:], 1.0)
            negh_col = singles.tile([P, 1], F32)    # -H (gpsimd chain const)
            nc.vector.memset(negh_col[:], -H_PARAM)
            dummy = singles.tile([P, 1], F32)
            nc.vector.memset(dummy[:], 0.0)
            mean_bf = singles.tile([P, C], BF16)
            # preload the Sqrt table set during the DMA phase (dep-free;
            # Square also lives in that set, so no reload before sq)
            nc.scalar.activation(out=dummy[:], in_=dummy[:], func=AF.Sqrt)

            nsy = singles.tile([P, NYT], F32)
            nsx = singles.tile([P, NXT], F32)
            sdy = singles.tile([P, NYT], F32)
            sdx = singles.tile([P, NXT], F32)
            invy = singles.tile([P, NYT], F32)
            nxh = singles.tile([P, NXT], F32)       # H*(1+eps)*nx
            scl_all = singles.tile([P, NRB], F32)   # c/nx per block
            gmax_all = singles.tile([P, NRB], F32)  # row max of G per block
            rq_all = singles.tile([P, NRB, 4], F32)  # quarter sums

            # ---- load inputs (y first: the mean gates everything).
            # 8-tile chunks = 4KB-contiguous per-partition descriptors (2KB
            # descriptors measurably cut DMA rate); y takes both HWDGE
            # queues first, x strictly after.  Chunk k on each queue covers
            # groups so the mean (which needs ALL of y) completes earliest.
            ysp = raw.tile([P, NYT, C], F32)
            xsp = raw.tile([P, NXT, C], F32)
            nc.sync.dma_start(out=ysp[:, 0:8, :], in_=fy_t[:, 0:8, :])
            nc.scalar.dma_start(out=ysp[:, 8:16, :], in_=fy_t[:, 8:16, :])
            nc.sync.dma_start(out=ysp[:, 16:24, :], in_=fy_t[:, 16:24, :])
            nc.scalar.dma_start(out=ysp[:, 24:32, :], in_=fy_t[:, 24:32, :])
            nc.sync.dma_start(out=xsp[:, 0:8, :], in_=fx_t[:, 0:8, :])
            nc.scalar.dma_start(out=xsp[:, 8:16, :], in_=fx_t[:, 8:16, :])

            # PE warmup: dep-free dummy matmuls while the DMA runs, so the
            # PE reaches full p-state before the first real block matmuls
            # (cold PE runs 512-wide matmuls ~2x slow).
            with tc.tile_pool(name="ps_warm", bufs=1,
                              space=bass.MemorySpace.PSUM) as warm_pool:
                warm = warm_pool.tile([P, CHUNK], F32)
                wsrc = singles.tile([P, CHUNK], BF16)
                nc.gpsimd.memset(wsrc[:], 0.0)
                for _ in range(24):
                    nc.tensor.matmul(warm[:], ones_bf[:], wsrc[:],
                                     start=True, stop=True)

            # bf16 casts per 8-tile DMA chunk (feed both the mean matmuls
            # and the centering chain)
            ybf_raw = raw.tile([P, NYT, C], BF16)
            xbf_raw = raw.tile([P, NXT, C], BF16)
            for h in range(NG):
                a = h * GT
                nc.vector.tensor_copy(ybf_raw[:, a:a + GT, :],
                                      ysp[:, a:a + GT, :])
            for j in range(2):
                nc.vector.tensor_copy(xbf_raw[:, j * 8:(j + 1) * 8, :],
                                      xsp[:, j * 8:(j + 1) * 8, :])

            # ---- mean over y's spatial axis: PE-accumulated from the bf16
            # casts as they land (1 cyc/row vs 2 for f32).  ones[P,P] @ tile
            # sums partitions; accumulating all 32 tiles sums tiles too.
            with tc.tile_pool(name="ps_bc", bufs=1,
                              space=bass.MemorySpace.PSUM) as ps_bc_pool:
                ps_bc = ps_bc_pool.tile([P, C], F32)
                for t in range(NYT):
                    nc.tensor.matmul(ps_bc[:], ones_bf[:], ybf_raw[:, t, :],
                                     start=(t == 0), stop=(t == NYT - 1))
                nc.scalar.mul(mean_bf[:], ps_bc[:], 1.0 / N)

            # second PE warmup batch: keeps the PE clocked up between the
            # mean matmuls and the first block matmuls (~20-31us window)
            with tc.tile_pool(name="ps_warm2", bufs=1,
                              space=bass.MemorySpace.PSUM) as warm_pool2:
                warm2 = warm_pool2.tile([P, CHUNK], F32)
                for _ in range(30):
                    nc.tensor.matmul(warm2[:], ones_bf[:], wsrc[:],
                                     start=True, stop=True)

            mean_g = mean_bf[:].rearrange("p (u c) -> p u c", u=1)

            def mbc(k):
                return mean_g.broadcast_to([P, k, C])

            # ---- y chain per 8-tile group: center, norm, normalize,
            # transpose.  All DVE ops are bf16 (2x mode).  The x subtract +
            # transpose slot in after group 0 (xt is needed by the first
            # block matmuls, but must not delay y group 0).
            xbf = raw.tile([P, NXT, C], BF16)
            sqx = raw.tile([P, NXT, C], F32)
            xt = tmats.tile([P, NXT, P], BF16)
            ybf = raw.tile([P, NYT, C], BF16)
            sq = raw.tile([P, NYT, C], F32)
            yt = tmats.tile([P, N], BF16)        # [c, (t q)] channel-major
            ytv = yt[:].rearrange("p (t q) -> p t q", q=P)
            ivg = invy[:].rearrange("p (t u) -> p t u", u=1)
            for g in range(NG):
                a = g * GT
                nc.vector.tensor_sub(ybf[:, a:a + GT, :],
                                     ybf_raw[:, a:a + GT, :], mbc(GT))
                nc.scalar.activation(out=sq[:, a:a + GT, :],
                                     in_=ybf[:, a:a + GT, :], func=AF.Square)
                nc.vector.reduce_sum(nsy[:, a:a + GT], sq[:, a:a + GT, :],
                                     axis=AX.X)
                nc.scalar.activation(sdy[:, a:a + GT], nsy[:, a:a + GT],
                                     AF.Sqrt)
                nc.vector.reciprocal(invy[:, a:a + GT], sdy[:, a:a + GT])
                # normalize on gpsimd: takes the biggest per-group op off
                # the DVE-serial critical chain (Pool is otherwise idle).
                # NOTE: keep ALL groups on one engine — splitting ybf's
                # writers across DVE and Pool produced an intermittent nan
                # on HW (and no speedup); mixed-engine writers to one tile
                # are not worth the risk.
                nc.gpsimd.tensor_mul(
                    ybf[:, a:a + GT, :], ybf[:, a:a + GT, :],
                    ivg[:, a:a + GT].broadcast_to([P, GT, C]))
                (nc.sync if g % 2 == 0 else nc.scalar).dma_start_transpose(
                    out=ytv[:, a:a + GT, :],
                    in_=ybf[:, a:a + GT, :].rearrange("p t c -> p (t c)"))
                if g == 0:
                    nc.vector.tensor_sub(xbf[:], xbf_raw[:], mbc(NXT))
                    nc.sync.dma_start_transpose(
                        out=xt[:],
                        in_=xbf[:].rearrange("p t c -> p (t c)"))

            # x norms after the y chain: they only feed the scale chain
            # (needed ~when block 0's maxes finish), and emitting them last
            # keeps the y-critical DVE stream unclogged.  The reduce is
            # split in 4 so no single 2.2us DVE op can wedge into the
            # y-critical window.
            nc.scalar.activation(out=sqx[:], in_=xbf[:], func=AF.Square)
            for j in range(4):
                nc.vector.reduce_sum(nsx[:, 4 * j:4 * (j + 1)],
                                     sqx[:, 4 * j:4 * (j + 1), :], axis=AX.X)
            nc.scalar.activation(sdx[:], nsx[:], AF.Sqrt)
            nc.vector.tensor_scalar_mul(nxh[:], sdx[:],
                                        H_PARAM * (1.0 + EPS_MIN))

            # preload the Exp activation table strictly after every Sqrt:
            # the dep tile mixes all four y-group sqrts and the x sqrt so
            # the scheduler cannot reorder any sqrt past the preload (a
            # reload on the loop's critical path costs 1.3us).
            sqdep = stat.tile([P, 1], F32, tag="sqdep", name="sqdep")
            nc.vector.tensor_add(sqdep[:], sdy[:, GT - 1:GT],
                                 sdy[:, 2 * GT - 1:2 * GT])
            for g in range(2, NG):
                nc.vector.tensor_add(sqdep[:], sqdep[:],
                                     sdy[:, (g + 1) * GT - 1:(g + 1) * GT])
            nc.vector.tensor_add(sqdep[:], sqdep[:], sdx[:, NXT - 1:NXT])
            # scale=0: exp(0)=1 regardless of the dep-sum's magnitude (the
            # sum of sqrts can overflow exp for finer group counts)
            nc.scalar.activation(out=dummy[:], in_=sqdep[:], func=AF.Exp,
                                 scale=0.0)

            def yrhs(j):       # 512-wide chunk j of the y matrix [C, N]
                return yt[:, CHUNK * j:CHUNK * (j + 1)]

            QUART = 1024
            NQ = N // QUART

            # ---- main loop: two interleaved passes with recompute
            # (pass1 max on PE+DVE, pass2 exp on PE+ACT, disjoint PSUM
            # halves, two blocks apart so the per-block scale chain
            # (gpsimd mul/add + DVE reciprocal) never stalls ACT).
            def pass1(rb, pool):
                lhs = xt[:, rb, :]
                mxq = stat.tile([P, NQ], F32, tag="mxq", name="mxq")
                for q in range(NQ):
                    ps = pool.tile([P, QUART], F32, tag="p1", name="ps1")
                    for j in range(2):
                        nc.tensor.matmul(
                            ps[:, j * CHUNK:(j + 1) * CHUNK],
                            lhs, yrhs(2 * q + j), start=True, stop=True)
                    nc.vector.reduce_max(mxq[:, q:q + 1], ps[:],
                                         axis=AX.X)
                gmax = gmax_all[:, rb:rb + 1]
                nc.vector.reduce_max(gmax, mxq[:], axis=AX.X)
                # tden = nxh - H*Gmax ; scl = 1/tden.  No bias: exp args
                # span only ~[-7, 7], so the max-shift is unnecessary;
                # the numerator exp(c*Smax) is applied once in the tail.
                tden = stat.tile([P, 1], F32, tag="tden", name="tden")
                if rb == 0:
                    nc.vector.tensor_scalar(
                        out=tden[:], in0=gmax, scalar1=-H_PARAM,
                        scalar2=nxh[:, rb:rb + 1],
                        op0=ALU.mult, op1=ALU.add)
                else:
                    hg = stat.tile([P, 1], F32, tag="hg", name="hg")
                    nc.gpsimd.tensor_mul(hg[:], gmax, negh_col[:])
                    nc.gpsimd.tensor_add(tden[:], hg[:], nxh[:, rb:rb + 1])
                nc.vector.reciprocal(scl_all[:, rb:rb + 1], tden[:])

            def p2mm(rb, pool, q):
                lhs = xt[:, rb, :]
                ps = pool.tile([P, QUART], F32, tag="p2", name="ps2")
                for j in range(2):
                    nc.tensor.matmul(
                        ps[:, j * CHUNK:(j + 1) * CHUNK],
                        lhs, yrhs(2 * q + j), start=True, stop=True)
                return ps

            def p2exp(rb, ps, q):
                nc.scalar.activation(
                    out=ps[:], in_=ps[:], func=AF.Exp,
                    scale=scl_all[:, rb:rb + 1],
                    accum_out=rq_all[:, rb, q:q + 1])

            def pass2(rb, pool, pre=()):
                for q, ps in pre:
                    p2exp(rb, ps, q)
                for q in range(len(pre), NQ):
                    ps = p2mm(rb, pool, q)
                    p2exp(rb, ps, q)

            with (
                tc.tile_pool(name="ps_p1", bufs=2,
                             space=bass.MemorySpace.PSUM) as pool1,
                tc.tile_pool(name="ps_p2", bufs=2,
                             space=bass.MemorySpace.PSUM) as pool2,
            ):
                # fill: hoist block 0's first two quarter-matmuls so the
                # first exp waits only on scl(0), not a third PE pass
                pass1(0, pool1)
                pre0 = [(q, p2mm(0, pool2, q)) for q in range(2)]
                pass1(1, pool1)
                for rb in range(2, NRB + 2):
                    pass2(rb - 2, pool2, pre=pre0 if rb == 2 else ())
                    if rb < NRB:
                        pass1(rb, pool1)

            # ---- tail: ship raw ingredients on three parallel queues ----
            nc.gpsimd.dma_start(
                out=part[:, 0:4 * NRB],
                in_=rq_all[:].rearrange("p b q -> p (b q)"))
            # gmax/scl ride the Sync queue: desc-gen on the idle Sync engine
            # issues as soon as block 15's scale chain lands (~4us before
            # the last accum read), so only the rq DMA's completion gates
            # the drain.  The scalar (ACT) queue would serialize desc-gen
            # behind the saturated ACT engine's last accumulator read.
            nc.sync.dma_start(out=part[:, 4 * NRB:5 * NRB], in_=gmax_all[:])
            nc.sync.dma_start(out=part[:, 5 * NRB:6 * NRB], in_=scl_all[:])

    nc.compile()
    return nc


_NC_CACHE = None


def _get_nc():
    global _NC_CACHE
    if _NC_CACHE is None:
        _NC_CACHE = build_nc()
    return _NC_CACHE


def _in_maps(feature_x, feature_y):
    fx = np.ascontiguousarray(
        np.asarray(feature_x, dtype=np.float32).reshape(B, N, C))
    fy = np.ascontiguousarray(
        np.asarray(feature_y, dtype=np.float32).reshape(B, N, C))
    maps = []
    for core in range(8):
        b, h = divmod(core, 2)
        maps.append({
            "fx": np.ascontiguousarray(fx[b, h * ROWS:(h + 1) * ROWS, :]),
            "fy": fy[b],
        })
    return maps


def _combine(results):
    # part[:, 0:64]=rq (per-block quarter exp-sums), [64:80]=Gmax,
    # [80:96]=scl.  maxA per block-row = exp(scl*Gmax) / sum_q rq.
    sums = []
    for r in results:
        part = np.asarray(r["part"], dtype=np.float64)
        rq = part[:, :4 * NRB].reshape(P, NRB, 4)
        gmax = part[:, 4 * NRB:5 * NRB]
        scl = part[:, 5 * NRB:6 * NRB]
        maxa = np.exp(scl * gmax) / rq.sum(axis=-1)
        sums.append(float(maxa.sum()))
    loss = np.empty(B, dtype=np.float64)
    for b in range(B):
        cx = (sums[2 * b] + sums[2 * b + 1]) / N
        loss[b] = -np.log(cx)
    return loss.astype(np.float32)


def kernel(feature_x, feature_y):
    nc = _get_nc()
    res = run_bass_kernel_spmd(nc, _in_maps(feature_x, feature_y),
                               core_ids=list(range(8)))
    return _combine(res.results)


def kernel_traced(feature_x, feature_y, **kwargs):
    """Like kernel() but with tracing; returns (loss, BassKernelResults)."""
    nc = _get_nc()
    res = run_bass_kernel_spmd(nc, _in_maps(feature_x, feature_y),
                               core_ids=list(range(8)), trace=True, **kwargs)
    return _combine(res.results), res
